# revision 6
# baseline (speedup 1.0000x reference)
"""Trainium2 Bass kernel for nn_Attention_41566693491235.

Computes, for full inputs (B=256, L=196, R=1024, A=512, D=2048):
    att_h  = h @ W_h + b_h                                  [B, A]
    dot    = einsum("bla,a->bl", tanh(f2 + att_h[:,None,:]), w_a) + b_a
    weight = softmax(dot, axis=1) * mask;  weight /= weight.sum(1, keepdims=True)
    att    = einsum("bl,bld->bd", weight, f1)               [B, D]

Sharding: data-parallel over batch, 32 per core x 8 cores. Weights replicated.
Note b_a and the softmax normalizer cancel exactly in the masked renorm:
    weight = exp(dot - max) * mask / sum(exp(dot - max) * mask)
"""

import numpy as np

import concourse.bass as bass
import concourse.bacc as bacc
import concourse.tile as tile
import concourse.mybir as mybir
from concourse import bass_utils

F32 = mybir.dt.float32
AF = mybir.ActivationFunctionType

# Problem shape (hardcoded; kernel.py must be self-contained).
B, L, R, A, D = 256, 196, 1024, 512, 2048
NCORES = 8
BL = B // NCORES          # 32 batches per core
NQ = BL // 4              # 8 quads of 4 batches
LFULL = (L // 32) * 32    # 192: l-range covered by full 32-row chunks
NLC = LFULL // 32         # 6 full l-chunks per quad
LRAG = L - LFULL          # 4: ragged l rows
KRAG = 4 * LRAG           # 16: ragged contraction rows (4 batches x 4 l)
NAC = A // 128            # 4 chunks of the attention-hidden dim
NKC = R // 128            # 8 chunks of the h-feature dim
NDC = D // 512            # 4 free-dim chunks for the output matmuls


def _build_program(f1_bufs: int = 12):
    nc = bacc.Bacc(
        "TRN2",
        target_bir_lowering=False,
        debug=False,
        enable_asserts=False,
        num_devices=NCORES,
    )

    hT = nc.dram_tensor("hT", [R, BL], F32, kind="ExternalInput").ap()
    wh = nc.dram_tensor("wh", [R, A], F32, kind="ExternalInput").ap()
    bh4 = nc.dram_tensor("bh4", [128, NAC], F32, kind="ExternalInput").ap()
    wa4 = nc.dram_tensor("wa4", [128, NAC], F32, kind="ExternalInput").ap()
    f2T = nc.dram_tensor("f2T", [BL, A, L], F32, kind="ExternalInput").ap()
    f1 = nc.dram_tensor("f1", [BL, L, D], F32, kind="ExternalInput").ap()
    msk = nc.dram_tensor("msk", [BL, L], F32, kind="ExternalInput").ap()
    bdm = nc.dram_tensor("bdm", [NQ, 128, BL], F32, kind="ExternalInput").ap()
    bdmr = nc.dram_tensor("bdmr", [NQ, KRAG, BL], F32, kind="ExternalInput").ap()
    att = nc.dram_tensor("att", [BL, D], F32, kind="ExternalOutput").ap()

    with tile.TileContext(nc) as tc:
        with (
            tc.tile_pool(name="const", bufs=1) as cpool,
            tc.tile_pool(name="f2p", bufs=3) as f2pool,
            tc.tile_pool(name="ep", bufs=3) as epool,
            tc.tile_pool(name="f1p", bufs=f1_bufs) as f1pool,
            tc.tile_pool(name="f1rp", bufs=2) as f1rpool,
            tc.tile_pool(name="small", bufs=1) as spool,
            tc.tile_pool(name="ps", bufs=1, space=bass.MemorySpace.PSUM) as pspool,
            tc.tile_pool(name="psdot", bufs=2, space=bass.MemorySpace.PSUM) as psdot,
            tc.tile_pool(name="dram", bufs=1, space=bass.MemorySpace.DRAM) as dpool,
        ):
            # ---- constants -------------------------------------------------
            wh_t = cpool.tile([128, NKC, A], F32)
            nc.sync.dma_start(wh_t[:], wh.rearrange("(kc p) a -> p kc a", p=128))
            hT_t = cpool.tile([128, NKC, BL], F32)
            nc.sync.dma_start(hT_t[:], hT.rearrange("(kc p) b -> p kc b", p=128))
            wa_t = cpool.tile([128, NAC], F32)
            nc.sync.dma_start(wa_t[:], wa4[:])
            bh_t = cpool.tile([128, NAC], F32)
            nc.sync.dma_start(bh_t[:], bh4[:])
            msk_t = cpool.tile([BL, L], F32)
            nc.sync.dma_start(msk_t[:], msk[:])
            bdm_t = cpool.tile([128, NQ, BL], F32)
            nc.sync.dma_start(bdm_t[:], bdm.rearrange("q k b -> k q b"))
            bdmr_t = cpool.tile([KRAG, NQ, BL], F32)
            nc.sync.dma_start(bdmr_t[:], bdmr.rearrange("q k b -> k q b"))

            # ---- phase 1: att_h.T = W_h.T @ h.T (+ b_h) --------------------
            # atth[:, ac, b] holds att_h[b, ac*128 + p] on partition p.
            ps_atth = pspool.tile([128, NAC, BL], F32)
            for mc in range(NAC):
                for kc in range(NKC):
                    nc.tensor.matmul(
                        ps_atth[:, mc, :],
                        wh_t[:, kc, mc * 128:(mc + 1) * 128],
                        hT_t[:, kc, :],
                        start=(kc == 0),
                        stop=(kc == NKC - 1),
                    )
            atth = cpool.tile([128, NAC, BL], F32)
            for mc in range(NAC):
                nc.vector.tensor_scalar_add(
                    atth[:, mc, :], ps_atth[:, mc, :], bh_t[:, mc:mc + 1]
                )

            # ---- phase 2: per-batch tanh + dot -----------------------------
            # dot[b, l] = sum_a tanh(f2[b,l,a] + att_h[b,a]) * w_a[a]
            dotflat = spool.tile([1, BL * L], F32)
            for b in range(BL):
                f2b = f2pool.tile([128, NAC, L], F32, tag="f2b")
                nc.sync.dma_start(
                    f2b[:], f2T[b].rearrange("(ac p) l -> p ac l", p=128)
                )
                eb = epool.tile([128, NAC, L], F32, tag="eb")
                for ac in range(NAC):
                    nc.scalar.activation(
                        eb[:, ac, :], f2b[:, ac, :], AF.Tanh,
                        bias=atth[:, ac, b:b + 1],
                    )
                pd = psdot.tile([1, L], F32, tag="pd")
                for ac in range(NAC):
                    nc.tensor.matmul(
                        pd[:], wa_t[:, ac:ac + 1], eb[:, ac, :],
                        start=(ac == 0), stop=(ac == NAC - 1),
                    )
                nc.vector.tensor_copy(dotflat[0:1, b * L:(b + 1) * L], pd[:])

            # ---- phase 3: batched masked softmax ---------------------------
            # Round-trip through DRAM to redistribute [1, BL*L] -> [BL, L].
            dot_dram = dpool.tile([1, BL * L], F32)
            nc.sync.dma_start(dot_dram[:], dotflat[:])
            dott = spool.tile([BL, L], F32)
            nc.sync.dma_start(dott[:], dot_dram.rearrange("o (b l) -> (o b) l", l=L))

            negmax = spool.tile([BL, 1], F32)
            nc.vector.tensor_reduce(
                negmax[:], dott[:], axis=mybir.AxisListType.X,
                op=mybir.AluOpType.max, negate=True,
            )
            wexp = spool.tile([BL, L], F32)
            nc.scalar.activation(wexp[:], dott[:], AF.Exp, bias=negmax[:])
            wm = spool.tile([BL, L], F32)
            nc.vector.tensor_mul(wm[:], wexp[:], msk_t[:])
            ssum = spool.tile([BL, 1], F32)
            nc.vector.reduce_sum(ssum[:], wm[:], axis=mybir.AxisListType.X)
            sinv = spool.tile([BL, 1], F32)
            nc.vector.reciprocal(sinv[:], ssum[:])
            wn = spool.tile([BL, L], F32)
            nc.vector.tensor_scalar_mul(wn[:], wm[:], sinv[:])

            # ---- phase 4: redistribute weights into block-diag layout ------
            # W2[bsub*32+j, q, lc] = wn[4q+bsub, 32*lc+j]. Stage through DRAM
            # in [q, lc, bs, j] order so both sides are legal rearranges.
            w_dram = dpool.tile([NQ, NLC, 4, 32], F32)
            w_dram_r = dpool.tile([NQ, 4, LRAG], F32)
            # wn rows iterate (q, bs); permute the DRAM out-AP to match.
            for q in range(NQ):
                nc.sync.dma_start(
                    w_dram[q].rearrange("lc bs j -> bs lc j"),
                    wn[4 * q:4 * q + 4, 0:LFULL],
                )
            nc.sync.dma_start(w_dram_r[:], wn[:, LFULL:L])
            w2 = spool.tile([128, NQ, NLC], F32)
            nc.sync.dma_start(w2[:], w_dram.rearrange("q lc bs j -> (bs j) q lc"))
            w2r = spool.tile([KRAG, NQ], F32)
            nc.sync.dma_start(w2r[:], w_dram_r.rearrange("q bs j -> (bs j) q"))
            # Block-diagonal stationary operands: ld[k, b'] = w strip iff b'
            # owns row k in this quad, else 0.
            ldt = spool.tile([128, NQ, NLC, BL], F32)
            for q in range(NQ):
                for lc in range(NLC):
                    nc.vector.tensor_scalar_mul(
                        ldt[:, q, lc, :], bdm_t[:, q, :], w2[:, q, lc:lc + 1]
                    )
            ldr = spool.tile([KRAG, NQ, BL], F32)
            for q in range(NQ):
                nc.vector.tensor_scalar_mul(
                    ldr[:, q, :], bdmr_t[:, q, :], w2r[:, q:q + 1]
                )

            # ---- phase 5: att = weight @ f1, 4 batches per matmul ----------
            ps_att = pspool.tile([BL, NDC, 512], F32)
            for q in range(NQ):
                for lc in range(NLC):
                    f1t = f1pool.tile([128, D], F32, tag="f1t")
                    # 3D DRAM AP into a [128, D] tile: dma only requires
                    # equal element counts and matching iteration order.
                    nc.sync.dma_start(
                        f1t[:], f1[4 * q:4 * q + 4, 32 * lc:32 * lc + 32, :]
                    )
                    for dc in range(NDC):
                        nc.tensor.matmul(
                            ps_att[:, dc, :],
                            ldt[:, q, lc, :],
                            f1t[:, dc * 512:(dc + 1) * 512],
                            start=(q == 0 and lc == 0),
                            stop=False,
                        )
                f1r = f1rpool.tile([KRAG, D], F32, tag="f1r")
                nc.sync.dma_start(f1r[:], f1[4 * q:4 * q + 4, LFULL:L, :])
                for dc in range(NDC):
                    nc.tensor.matmul(
                        ps_att[:, dc, :],
                        ldr[:, q, :],
                        f1r[:, dc * 512:(dc + 1) * 512],
                        start=False,
                        stop=(q == NQ - 1),
                    )

            att_sb = spool.tile([BL, D], F32)
            nc.vector.tensor_copy(
                att_sb[:], ps_att.rearrange("b dc n -> b (dc n)")
            )
            nc.sync.dma_start(att[:], att_sb[:])

    nc.compile()
    return nc


_PROGRAM_CACHE = {}


def _get_program():
    if "nc" not in _PROGRAM_CACHE:
        _PROGRAM_CACHE["nc"] = _build_program()
    return _PROGRAM_CACHE["nc"]


def _block_diag_masks():
    bdm = np.zeros((NQ, 128, BL), dtype=np.float32)
    bdmr = np.zeros((NQ, KRAG, BL), dtype=np.float32)
    for q in range(NQ):
        for bs in range(4):
            bdm[q, bs * 32:(bs + 1) * 32, 4 * q + bs] = 1.0
            bdmr[q, bs * LRAG:(bs + 1) * LRAG, 4 * q + bs] = 1.0
    return bdm, bdmr


def make_in_maps(h, att_feats1, att_feats2, att_masks, W_h, b_h, w_a, b_a):
    h = np.asarray(h, dtype=np.float32)
    att_feats1 = np.asarray(att_feats1, dtype=np.float32)
    att_feats2 = np.asarray(att_feats2, dtype=np.float32)
    att_masks = np.asarray(att_masks, dtype=np.float32)
    W_h = np.ascontiguousarray(np.asarray(W_h, dtype=np.float32))
    b_h = np.asarray(b_h, dtype=np.float32)
    w_a = np.asarray(w_a, dtype=np.float32)
    del b_a  # cancels exactly in the softmax + masked renormalization

    wa4 = np.ascontiguousarray(w_a.reshape(NAC, 128).T)
    bh4 = np.ascontiguousarray(b_h.reshape(NAC, 128).T)
    bdm, bdmr = _block_diag_masks()

    in_maps = []
    for c in range(NCORES):
        sl = slice(c * BL, (c + 1) * BL)
        in_maps.append({
            "hT": np.ascontiguousarray(h[sl].T),
            "wh": W_h,
            "bh4": bh4,
            "wa4": wa4,
            "f2T": np.ascontiguousarray(att_feats2[sl].transpose(0, 2, 1)),
            "f1": np.ascontiguousarray(att_feats1[sl]),
            "msk": np.ascontiguousarray(att_masks[sl]),
            "bdm": bdm,
            "bdmr": bdmr,
        })
    return in_maps


def kernel(h, att_feats1, att_feats2, att_masks, W_h, b_h, w_a, b_a,
           _trace=False, _return_results=False):
    nc = _get_program()
    in_maps = make_in_maps(h, att_feats1, att_feats2, att_masks, W_h, b_h,
                           w_a, b_a)
    res = bass_utils.run_bass_kernel_spmd(
        nc, in_maps, core_ids=list(range(NCORES)), trace=_trace
    )
    out = np.concatenate([res.results[c]["att"] for c in range(NCORES)], axis=0)
    if _return_results:
        return out, res
    return out


# revision 10
# speedup vs baseline: 2.3098x; 2.3098x over previous
"""Trainium2 Bass kernel for nn_Attention_41566693491235.

Computes, for full inputs (B=256, L=196, R=1024, A=512, D=2048):
    att_h  = h @ W_h + b_h                                  [B, A]
    dot    = einsum("bla,a->bl", tanh(f2 + att_h[:,None,:]), w_a) + b_a
    weight = softmax(dot, axis=1) * mask;  weight /= weight.sum(1, keepdims=True)
    att    = einsum("bl,bld->bd", weight, f1)               [B, D]

Sharding: data-parallel over batch, 32 per core x 8 cores. Weights replicated.
Note b_a and the softmax normalizer cancel exactly in the masked renorm:
    weight = exp(dot - max) * mask / sum(exp(dot - max) * mask)

Implementation notes:
  - Matmuls run in float32r (full-rate PE at moving-dim >= 256, ~2e-4 rel err).
  - The final weighted sum packs 4 batches per 128-row contraction via
    block-diagonal stationary operands, so all 32 output rows accumulate
    partition-aligned in one PSUM tile. Contraction rows use k = j*4 + bs
    (j = l-offset, bs = batch-in-quad) so the f1 DMAs have a 32-entry outer
    dim and fan out across all 16 DMA engines.
"""

import numpy as np

import concourse.bass as bass
import concourse.bacc as bacc
import concourse.tile as tile
import concourse.mybir as mybir
from concourse import bass_utils

F32 = mybir.dt.float32
F32R = mybir.dt.float32r
AF = mybir.ActivationFunctionType

# Problem shape (hardcoded; kernel.py must be self-contained).
B, L, R, A, D = 256, 196, 1024, 512, 2048
NCORES = 8
BL = B // NCORES          # 32 batches per core
NQ = BL // 4              # 8 quads of 4 batches
LFULL = (L // 32) * 32    # 192: l-range covered by full 32-row chunks
NLC = LFULL // 32         # 6 full l-chunks per quad
LRAG = L - LFULL          # 4: ragged l rows
KRAG = 4 * LRAG           # 16: ragged contraction rows (4 l x 4 batches)
NAC = A // 128            # 4 chunks of the attention-hidden dim
NKC = R // 128            # 8 chunks of the h-feature dim
NDC = D // 512            # 4 free-dim chunks for the output matmuls


def _build_program(f1_bufs: int = 12):
    nc = bacc.Bacc(
        "TRN2",
        target_bir_lowering=False,
        debug=False,
        enable_asserts=False,
        num_devices=NCORES,
    )

    hT = nc.dram_tensor("hT", [R, BL], F32, kind="ExternalInput").ap()
    wh = nc.dram_tensor("wh", [R, A], F32, kind="ExternalInput").ap()
    bh4 = nc.dram_tensor("bh4", [128, NAC], F32, kind="ExternalInput").ap()
    wa4 = nc.dram_tensor("wa4", [128, NAC], F32, kind="ExternalInput").ap()
    f2T = nc.dram_tensor("f2T", [BL, A, L], F32, kind="ExternalInput").ap()
    f1 = nc.dram_tensor("f1", [BL, L, D], F32, kind="ExternalInput").ap()
    msk = nc.dram_tensor("msk", [BL, L], F32, kind="ExternalInput").ap()
    bdm = nc.dram_tensor("bdm", [NQ, 128, BL], F32, kind="ExternalInput").ap()
    bdmr = nc.dram_tensor("bdmr", [NQ, KRAG, BL], F32, kind="ExternalInput").ap()
    att = nc.dram_tensor("att", [BL, D], F32, kind="ExternalOutput").ap()

    with tile.TileContext(nc) as tc:
        with (
            tc.tile_pool(name="const", bufs=1) as cpool,
            tc.tile_pool(name="f2p", bufs=3) as f2pool,
            tc.tile_pool(name="ep", bufs=3) as epool,
            tc.tile_pool(name="f1p", bufs=f1_bufs) as f1pool,
            tc.tile_pool(name="f1rp", bufs=1) as f1rpool,
            tc.tile_pool(name="small", bufs=1) as spool,
            tc.tile_pool(name="ps", bufs=1, space=bass.MemorySpace.PSUM) as pspool,
            tc.tile_pool(name="psdot", bufs=2, space=bass.MemorySpace.PSUM) as psdot,
            tc.tile_pool(name="dram", bufs=1, space=bass.MemorySpace.DRAM) as dpool,
        ):
            # ---- constants -------------------------------------------------
            wh_t = cpool.tile([128, NKC, A], F32R)
            nc.sync.dma_start(wh_t[:], wh.bitcast(F32R).rearrange("(kc p) a -> p kc a", p=128))
            hT_t = cpool.tile([128, NKC, BL], F32R)
            nc.sync.dma_start(hT_t[:], hT.bitcast(F32R).rearrange("(kc p) b -> p kc b", p=128))
            wa_t = cpool.tile([128, NAC], F32R)
            nc.sync.dma_start(wa_t[:], wa4.bitcast(F32R)[:])
            bh_t = cpool.tile([128, NAC], F32)
            nc.sync.dma_start(bh_t[:], bh4[:])
            msk_t = cpool.tile([BL, L], F32)
            nc.sync.dma_start(msk_t[:], msk[:])
            bdm_t = cpool.tile([128, NQ, BL], F32)
            nc.sync.dma_start(bdm_t[:], bdm.rearrange("q k b -> k q b"))
            bdmr_t = cpool.tile([KRAG, NQ, BL], F32)
            nc.sync.dma_start(bdmr_t[:], bdmr.rearrange("q k b -> k q b"))

            # ---- phase 1: att_h.T = W_h.T @ h.T (+ b_h) --------------------
            # atth[:, ac, b] holds att_h[b, ac*128 + p] on partition p.
            ps_atth = pspool.tile([128, NAC, BL], F32)
            for mc in range(NAC):
                for kc in range(NKC):
                    nc.tensor.matmul(
                        ps_atth[:, mc, :],
                        wh_t[:, kc, mc * 128:(mc + 1) * 128],
                        hT_t[:, kc, :],
                        start=(kc == 0),
                        stop=(kc == NKC - 1),
                    )
            atth = cpool.tile([128, NAC, BL], F32)
            for mc in range(NAC):
                nc.vector.tensor_scalar_add(
                    atth[:, mc, :], ps_atth[:, mc, :], bh_t[:, mc:mc + 1]
                )

            # ---- phase 2: tanh + dot, two batches per matmul ---------------
            # dot[b, l] = sum_a tanh(f2[b,l,a] + att_h[b,a]) * w_a[a]
            dotflat = spool.tile([1, BL * L], F32)
            for bp in range(BL // 2):
                f2b = f2pool.tile([128, 2, NAC, L], F32, tag="f2b")
                for i in range(2):
                    nc.sync.dma_start(
                        f2b[:, i], f2T[2 * bp + i].rearrange("(ac p) l -> p ac l", p=128)
                    )
                e2 = epool.tile([128, NAC, 2, L], F32R, tag="e2")
                for i in range(2):
                    for ac in range(NAC):
                        nc.scalar.activation(
                            e2[:, ac, i, :], f2b[:, i, ac, :], AF.Tanh,
                            bias=atth[:, ac, 2 * bp + i:2 * bp + i + 1],
                        )
                pd = psdot.tile([1, 2, L], F32, tag="pd")
                for ac in range(NAC):
                    nc.tensor.matmul(
                        pd[:], wa_t[:, ac:ac + 1], e2[:, ac, :, :],
                        start=(ac == 0), stop=(ac == NAC - 1),
                    )
                nc.vector.tensor_copy(
                    dotflat[0:1, 2 * bp * L:(2 * bp + 2) * L], pd[:]
                )

            # ---- phase 3: batched masked softmax ---------------------------
            # Round-trip through DRAM to redistribute [1, BL*L] -> [BL, L].
            dot_dram = dpool.tile([1, BL * L], F32)
            nc.sync.dma_start(dot_dram[:], dotflat[:])
            dott = spool.tile([BL, L], F32)
            nc.sync.dma_start(dott[:], dot_dram.rearrange("o (b l) -> (o b) l", l=L))

            negmax = spool.tile([BL, 1], F32)
            nc.vector.tensor_reduce(
                negmax[:], dott[:], axis=mybir.AxisListType.X,
                op=mybir.AluOpType.max, negate=True,
            )
            wexp = spool.tile([BL, L], F32)
            nc.scalar.activation(wexp[:], dott[:], AF.Exp, bias=negmax[:])
            wm = spool.tile([BL, L], F32)
            nc.vector.tensor_mul(wm[:], wexp[:], msk_t[:])
            ssum = spool.tile([BL, 1], F32)
            nc.vector.reduce_sum(ssum[:], wm[:], axis=mybir.AxisListType.X)
            sinv = spool.tile([BL, 1], F32)
            nc.vector.reciprocal(sinv[:], ssum[:])
            wn = spool.tile([BL, L], F32)
            nc.vector.tensor_scalar_mul(wn[:], wm[:], sinv[:])

            # ---- phase 4: redistribute weights into block-diag layout ------
            # W2[j*4+bs, q, lc] = wn[4q+bs, 32*lc+j]. Stage through DRAM in
            # [q, lc, j, bs] order so both sides are legal <=3-dim APs.
            w_dram = dpool.tile([NQ, NLC, 32, 4], F32)
            w_dram_r = dpool.tile([NQ, LRAG, 4], F32)
            for q in range(NQ):
                nc.sync.dma_start(
                    w_dram[q].rearrange("lc j bs -> bs lc j"),
                    wn[4 * q:4 * q + 4, 0:LFULL],
                )
                nc.sync.dma_start(
                    w_dram_r[q].rearrange("j bs -> bs j"),
                    wn[4 * q:4 * q + 4, LFULL:L],
                )
            w2 = spool.tile([128, NQ, NLC], F32)
            nc.sync.dma_start(w2[:], w_dram.rearrange("q lc j bs -> (j bs) q lc"))
            w2r = spool.tile([KRAG, NQ], F32)
            nc.sync.dma_start(w2r[:], w_dram_r.rearrange("q j bs -> (j bs) q"))
            # Block-diagonal stationary operands: ld[k, b'] = w strip iff b'
            # owns row k in this quad, else 0.
            ldt = spool.tile([128, NQ, NLC, BL], F32R)
            for q in range(NQ):
                for lc in range(NLC):
                    nc.vector.tensor_scalar_mul(
                        ldt[:, q, lc, :], bdm_t[:, q, :], w2[:, q, lc:lc + 1]
                    )
            ldr = spool.tile([KRAG, NQ, BL], F32R)
            for q in range(NQ):
                nc.vector.tensor_scalar_mul(
                    ldr[:, q, :], bdmr_t[:, q, :], w2r[:, q:q + 1]
                )

            # ---- phase 5: att = weight @ f1, 4 batches per matmul ----------
            ps_att = pspool.tile([BL, NDC, 512], F32)
            for q in range(NQ):
                for lc in range(NLC):
                    f1t = f1pool.tile([128, D], F32R, tag="f1t")
                    # DRAM AP [32(j), 4(bs), 2048]: 32-entry outer dim fans
                    # the transfer across all 16 DMA engines.
                    nc.sync.dma_start(
                        f1t[:],
                        f1.bitcast(F32R)[
                            4 * q:4 * q + 4, 32 * lc:32 * lc + 32, :
                        ].rearrange("bs j d -> j bs d"),
                    )
                    for dc in range(NDC):
                        nc.tensor.matmul(
                            ps_att[:, dc, :],
                            ldt[:, q, lc, :],
                            f1t[:, dc * 512:(dc + 1) * 512],
                            start=(q == 0 and lc == 0),
                            stop=False,
                        )
                f1r = f1rpool.tile([KRAG, D], F32R, tag="f1r")
                nc.sync.dma_start(
                    f1r[:],
                    f1.bitcast(F32R)[4 * q:4 * q + 4, LFULL:L, :].rearrange(
                        "bs j d -> j bs d"
                    ),
                )
                for dc in range(NDC):
                    nc.tensor.matmul(
                        ps_att[:, dc, :],
                        ldr[:, q, :],
                        f1r[:, dc * 512:(dc + 1) * 512],
                        start=False,
                        stop=(q == NQ - 1),
                    )

            for dc in range(NDC):
                att_sb = spool.tile([BL, 512], F32, tag="att_sb", bufs=2)
                nc.vector.tensor_copy(att_sb[:], ps_att[:, dc, :])
                nc.sync.dma_start(att[:, dc * 512:(dc + 1) * 512], att_sb[:])

    nc.compile()
    return nc


_PROGRAM_CACHE = {}


def _get_program():
    if "nc" not in _PROGRAM_CACHE:
        _PROGRAM_CACHE["nc"] = _build_program()
    return _PROGRAM_CACHE["nc"]


def _block_diag_masks():
    bdm = np.zeros((NQ, 128, BL), dtype=np.float32)
    bdmr = np.zeros((NQ, KRAG, BL), dtype=np.float32)
    for q in range(NQ):
        for bs in range(4):
            bdm[q, bs::4, 4 * q + bs] = 1.0        # rows k = j*4 + bs
            bdmr[q, bs::4, 4 * q + bs] = 1.0
    return bdm, bdmr


def make_in_maps(h, att_feats1, att_feats2, att_masks, W_h, b_h, w_a, b_a):
    h = np.asarray(h, dtype=np.float32)
    att_feats1 = np.asarray(att_feats1, dtype=np.float32)
    att_feats2 = np.asarray(att_feats2, dtype=np.float32)
    att_masks = np.asarray(att_masks, dtype=np.float32)
    W_h = np.ascontiguousarray(np.asarray(W_h, dtype=np.float32))
    b_h = np.asarray(b_h, dtype=np.float32)
    w_a = np.asarray(w_a, dtype=np.float32)
    del b_a  # cancels exactly in the softmax + masked renormalization

    wa4 = np.ascontiguousarray(w_a.reshape(NAC, 128).T)
    bh4 = np.ascontiguousarray(b_h.reshape(NAC, 128).T)
    bdm, bdmr = _block_diag_masks()

    in_maps = []
    for c in range(NCORES):
        sl = slice(c * BL, (c + 1) * BL)
        in_maps.append({
            "hT": np.ascontiguousarray(h[sl].T),
            "wh": W_h,
            "bh4": bh4,
            "wa4": wa4,
            "f2T": np.ascontiguousarray(att_feats2[sl].transpose(0, 2, 1)),
            "f1": np.ascontiguousarray(att_feats1[sl]),
            "msk": np.ascontiguousarray(att_masks[sl]),
            "bdm": bdm,
            "bdmr": bdmr,
        })
    return in_maps


def kernel(h, att_feats1, att_feats2, att_masks, W_h, b_h, w_a, b_a,
           _trace=False, _return_results=False):
    nc = _get_program()
    in_maps = make_in_maps(h, att_feats1, att_feats2, att_masks, W_h, b_h,
                           w_a, b_a)
    res = bass_utils.run_bass_kernel_spmd(
        nc, in_maps, core_ids=list(range(NCORES)), trace=_trace
    )
    out = np.concatenate([res.results[c]["att"] for c in range(NCORES)], axis=0)
    if _return_results:
        return out, res
    return out
